# revision 18
# baseline (speedup 1.0000x reference)
"""Trainium2 Bass kernel for nn_ConvLTVFilterGenerator (v2).

Pipeline (per batch elem, data-parallel over B across 8 cores, 2 elems/core):
  conv stack (4 conv1d k=3 layers, grouped convs as block-diag halves)
  -> cepstrum DFT (matmul vs cos/sin matrices, quef folded into W4)
  -> Z-1 ~= u + i*phi (1st-order Taylor; |u|,|phi| < 0.011 so the 2nd-order
     term is ~1e-5 relative)
  -> P = (Z-1) * conj(F) per frame, F = frame DFT via bf16 matmuls
  -> window + OLA fused into the final matmul (PSUM accumulates the t and
     t-1 halves); k=512 Nyquist bin and the identity-delta path dropped
     (together ~6e-3 relative, budget 2e-2)

Engine split per spectral iteration (kc, t0): PE 12 matmuls (4800 cyc),
Act 4 PSUM->SBUF bf16 copies, DVE 6 bf16 product ops. PE-bound by design.
"""
import sys

sys.path.insert(0, "/opt/trn_rl_repo")

import numpy as np
import ml_dtypes

import concourse.bacc as bacc
import concourse.tile as tile
from concourse import mybir
from concourse.bass_utils import run_bass_kernel_spmd

B, T, D = 16, 800, 80
HOP, WIN, FFT = 256, 512, 1024
CCH, OUT, GRP = 256, 222, 8
NB = 512                   # spectral bins kept (Nyquist dropped)
N_CORES = 8
BPC = B // N_CORES         # 2 batch elems per core
ZPAD = T * HOP + 512       # 205312 = 1604*128
NU = ZPAD // 128           # 1604
F = 400                    # frames per matmul half

f32 = mybir.dt.float32
f32r = mybir.dt.float32r
bf16 = mybir.dt.bfloat16
AF = mybir.ActivationFunctionType
ALU = mybir.AluOpType
BF = ml_dtypes.bfloat16

_MATS = None
_NCS = {}


def _build_matrices():
    """Input-independent DFT/OLA matrices, host-side fp64 -> fp32/bf16."""
    global _MATS
    if _MATS is not None:
        return _MATS
    w = 2 * np.pi / FFT
    k = np.arange(NB, dtype=np.float64)[:, None]          # (512, 1)
    c = np.arange(OUT, dtype=np.float64)[None, :]
    s_exp = np.log(10.0) / 10.0
    pad = (FFT - OUT) // 2
    CaN = np.cos(w * k * (pad + c)) * s_exp               # (512, 222)
    SaN = -np.sin(w * k * (pad + c))
    j = np.arange(WIN, dtype=np.float64)[None, :]
    C1 = np.cos(w * k * j)                                # (512, 512)
    S1 = -np.sin(w * k * j)
    n = np.arange(WIN, dtype=np.float64)
    win = 0.5 * (1.0 - np.cos(2.0 * np.pi * n / WIN))
    wk = np.full(NB, 2.0); wk[0] = 1.0
    d = (WIN - 1 - n)[None, :]
    C2 = (win[None, :] * wk[:, None] * np.cos(w * k * d)) / FFT   # (512, 512)
    S2 = (-win[None, :] * wk[:, None] * np.sin(w * k * d)) / FFT

    cat = np.zeros((128, 2, NB), np.float64)
    sat = np.zeros((128, 2, NB), np.float64)
    for ch in range(2):
        rows = min(128, OUT - 128 * ch)
        cat[:rows, ch, :] = CaN[:, 128 * ch:128 * ch + rows].T
        sat[:rows, ch, :] = SaN[:, 128 * ch:128 * ch + rows].T
    c1t = np.zeros((128, 4, NB), np.float64)
    s1t = np.zeros((128, 4, NB), np.float64)
    for a in range(4):
        c1t[:, a, :] = C1[:, 128 * a:128 * (a + 1)].T
        s1t[:, a, :] = S1[:, 128 * a:128 * (a + 1)].T
    c2a = np.zeros((128, 4, HOP), np.float64)
    c2b = np.zeros((128, 4, HOP), np.float64)
    s2a = np.zeros((128, 4, HOP), np.float64)
    s2b = np.zeros((128, 4, HOP), np.float64)
    for kc in range(4):
        c2a[:, kc, :] = C2[128 * kc:128 * (kc + 1), :HOP]
        c2b[:, kc, :] = C2[128 * kc:128 * (kc + 1), HOP:]
        s2a[:, kc, :] = S2[128 * kc:128 * (kc + 1), :HOP]
        s2b[:, kc, :] = S2[128 * kc:128 * (kc + 1), HOP:]

    def f32a(a):
        return np.ascontiguousarray(a, np.float32)

    def bfa(a):
        return np.ascontiguousarray(np.asarray(a, np.float32).astype(BF))

    _MATS = dict(
        cat=f32a(cat), sat=f32a(sat),
        c1t=bfa(c1t), s1t=bfa(s1t),
        c2a=bfa(c2a), c2b=bfa(c2b), s2a=bfa(s2a), s2b=bfa(s2b))
    return _MATS


def _prep_weights(inp):
    """Input-dependent weight rearrangements (host)."""
    idx = np.arange(1, OUT // 2 + 1, dtype=np.float64)
    quef = np.concatenate([idx[::-1], idx])
    W1 = np.asarray(inp["W1"], np.float64)
    W2 = np.asarray(inp["W2"], np.float64)
    W3 = np.asarray(inp["W3"], np.float64)
    W4 = np.asarray(inp["W4"], np.float64)
    # conv1 tap-packed: contraction row R = k*80+c (k tap, c in-channel),
    # split into two K=120 chunks
    w1s = np.zeros((120, 2, CCH), np.float64)
    for R in range(240):
        k, cin = R // D, R % D
        w1s[R % 120, R // 120, :] = W1[:, cin, k]

    def blockdiag(W):
        bd = np.zeros((128, 3, 2, 128), np.float64)
        for H in range(2):
            for o in range(128):
                g = o // 32
                for kk in range(3):
                    bd[g * 32:(g + 1) * 32, kk, H, o] = W[128 * H + o, :, kk]
        return np.ascontiguousarray(bd, np.float32)

    W4q = W4 / quef[:, None, None]
    w4t = np.zeros((128, 2, 3, OUT), np.float64)
    for cch in range(2):
        for kk in range(3):
            w4t[:, cch, kk, :] = W4q[:, 128 * cch:128 * (cch + 1), kk].T
    b4q = np.asarray(inp["b4"], np.float64) / quef

    def bias2(b):
        out = np.zeros((128, 2), np.float32)
        bb = np.asarray(b, np.float64)
        out[:, 0] = bb[:128]
        out[:len(bb) - 128, 1] = bb[128:]
        return out

    return dict(
        w1s=np.ascontiguousarray(w1s, np.float32),
        bd2=blockdiag(W2), bd3=blockdiag(W3),
        w4t=np.ascontiguousarray(w4t, np.float32),
        b1t=bias2(inp["b1"]), b2t=bias2(inp["b2"]), b3t=bias2(inp["b3"]),
        b4t=bias2(b4q))


def build_nc(loop_n=1):
    """Build + compile the per-core Bass program."""
    if loop_n in _NCS:
        return _NCS[loop_n]
    nc = bacc.Bacc("TRN2", target_bir_lowering=False, debug=False)

    def din(name, shape, dt=f32r):
        return nc.dram_tensor(name, list(shape), dt, kind="ExternalInput").ap()

    XS = din("xs", (BPC, 2, 120, T))
    VT = din("vt", (BPC, 128, NU), bf16)
    CAT = din("cat", (128, 2, NB)); SAT = din("sat", (128, 2, NB))
    C1T = din("c1t", (128, 4, NB), bf16); S1T = din("s1t", (128, 4, NB), bf16)
    C2A = din("c2a", (128, 4, HOP), bf16); C2B = din("c2b", (128, 4, HOP), bf16)
    S2A = din("s2a", (128, 4, HOP), bf16); S2B = din("s2b", (128, 4, HOP), bf16)
    W1S = din("w1s", (120, 2, CCH))
    BD2 = din("bd2", (128, 3, 2, 128)); BD3 = din("bd3", (128, 3, 2, 128))
    W4T = din("w4t", (128, 2, 3, OUT))
    B1 = nc.dram_tensor("b1t", [128, 2], f32, kind="ExternalInput").ap()
    B2 = nc.dram_tensor("b2t", [128, 2], f32, kind="ExternalInput").ap()
    B3 = nc.dram_tensor("b3t", [128, 2], f32, kind="ExternalInput").ap()
    B4 = nc.dram_tensor("b4t", [128, 2], f32, kind="ExternalInput").ap()
    OUTD = nc.dram_tensor("out", [BPC, T, HOP], f32, kind="ExternalOutput").ap()

    with tile.TileContext(nc) as tc:
        with tc.tile_pool(name="consts", bufs=1) as cst, \
             tc.tile_pool(name="work", bufs=2) as wk, \
             tc.tile_pool(name="psc", bufs=2, space="PSUM") as psc, \
             tc.tile_pool(name="pss", bufs=2, space="PSUM") as pss, \
             tc.tile_pool(name="pso", bufs=2, space="PSUM") as pso:

            def load(name, src, shape, dt=f32r):
                t = cst.tile(list(shape), dt, name=name)
                nc.sync.dma_start(out=t, in_=src)
                return t

            cat = load("catS", CAT, (128, 2, NB))
            sat = load("satS", SAT, (128, 2, NB))
            c1t = load("c1tS", C1T, (128, 4, NB), bf16)
            s1t = load("s1tS", S1T, (128, 4, NB), bf16)
            c2a = load("c2aS", C2A, (128, 4, HOP), bf16)
            c2b = load("c2bS", C2B, (128, 4, HOP), bf16)
            s2a = load("s2aS", S2A, (128, 4, HOP), bf16)
            s2b = load("s2bS", S2B, (128, 4, HOP), bf16)
            w1s = load("w1sS", W1S, (120, 2, CCH))
            bd2 = load("bd2S", BD2, (128, 3, 2, 128))
            bd3 = load("bd3S", BD3, (128, 3, 2, 128))
            w4t = load("w4tS", W4T, (128, 2, 3, OUT))
            b1t = load("b1tS", B1, (128, 2), f32)
            b2t = load("b2tS", B2, (128, 2), f32)
            b3t = load("b3tS", B3, (128, 2), f32)
            b4t = load("b4tS", B4, (128, 2), f32)

            # persistent data tiles, manually double-buffered on dim 1
            xs_sb = cst.tile([120, BPC, 2, T], f32r, name="xs_sb")
            V = cst.tile([128, BPC, NU], bf16, name="V")
            h1 = cst.tile([128, 2, T + 2], f32r, name="h1")
            h2 = cst.tile([128, 2, T + 2], f32r, name="h2")
            h3 = cst.tile([128, 2, T + 2], f32r, name="h3")
            ccep = cst.tile([128, BPC, 2, T], f32r, name="ccep")
            pre = cst.tile([128, 4, BPC, T + 1], bf16, name="pre")
            pim = cst.tile([128, 4, BPC, T + 1], bf16, name="pim")
            # zero the conv halo columns once (never rewritten)
            zb = cst.tile([128, 1], f32, name="zb")
            nc.vector.memset(zb, 0.0)
            for h in (h1, h2, h3):
                for m in range(2):
                    nc.vector.tensor_copy(h[:, m, 0:1], zb)
                    nc.vector.tensor_copy(h[:, m, T + 1:T + 2], zb)

            def load_b(b):
                for ch in range(2):
                    nc.sync.dma_start(out=xs_sb[:, b, ch, :], in_=XS[b, ch])
                nc.sync.dma_start(out=V[:, b, :], in_=VT[b])

            def relu_psum(dst, pc, bt, on_act):
                """bias-add + relu from PSUM, on Act or DVE."""
                if on_act:
                    nc.scalar.activation(dst, pc, AF.Relu, bias=bt, scale=1.0)
                else:
                    nc.vector.tensor_scalar(dst, pc, bt, 0.0, ALU.add, ALU.max)

            def conv(b):
                for t0 in (0, F):
                    for m in range(2):
                        pc = psc.tile([128, F], f32, tag="pc", name="pc1")
                        for ch in range(2):
                            nc.tensor.matmul(
                                pc, w1s[:, ch, 128 * m:128 * (m + 1)],
                                xs_sb[:, b, ch, t0:t0 + F],
                                start=(ch == 0), stop=(ch == 1))
                        relu_psum(h1[:, m, 1 + t0:1 + t0 + F], pc,
                                  b1t[:, m:m + 1], on_act=(m == 0))
                for hsrc, hdst, bdw, bt in ((h1, h2, bd2, b2t),
                                            (h2, h3, bd3, b3t)):
                    for t0 in (0, F):
                        for m in range(2):
                            pc = psc.tile([128, F], f32, tag="pc", name="pc2")
                            for kk in range(3):
                                nc.tensor.matmul(
                                    pc, bdw[:, kk, m, :],
                                    hsrc[:, m, t0 + kk:t0 + kk + F],
                                    start=(kk == 0), stop=(kk == 2))
                            relu_psum(hdst[:, m, 1 + t0:1 + t0 + F], pc,
                                      bt[:, m:m + 1], on_act=(m == 0))
                for t0 in (0, F):
                    for m in range(2):
                        sz = min(128, OUT - 128 * m)
                        pc = psc.tile([128, F], f32, tag="pc", name="pc4")
                        first = True
                        for cch in range(2):
                            for kk in range(3):
                                nc.tensor.matmul(
                                    pc[:sz], w4t[:, cch, kk, 128 * m:128 * m + sz],
                                    h3[:, cch, t0 + kk:t0 + kk + F],
                                    start=first, stop=(cch == 1 and kk == 2))
                                first = False
                        nc.vector.tensor_scalar_add(
                            ccep[:sz, b, m, t0:t0 + F], pc[:sz],
                            b4t[:sz, m:m + 1])

            def spectral(b):
                for t0 in (0, F):
                    for kc in range(4):
                        ks = slice(128 * kc, 128 * (kc + 1))
                        rey = pss.tile([128, F], f32, tag="ri", name="rey")
                        nc.tensor.matmul(rey, cat[:, 0, ks],
                                         ccep[:, b, 0, t0:t0 + F],
                                         start=True, stop=False)
                        nc.tensor.matmul(rey, cat[:94, 1, ks],
                                         ccep[:94, b, 1, t0:t0 + F],
                                         start=False, stop=True)
                        imy = pss.tile([128, F], f32, tag="ri", name="imy")
                        nc.tensor.matmul(imy, sat[:, 0, ks],
                                         ccep[:, b, 0, t0:t0 + F],
                                         start=True, stop=False)
                        nc.tensor.matmul(imy, sat[:94, 1, ks],
                                         ccep[:94, b, 1, t0:t0 + F],
                                         start=False, stop=True)
                        fr = pss.tile([128, F], f32, tag="ff", name="fr")
                        for a in range(4):
                            rhs = V[:, b, 2 * t0 + a:2 * (t0 + F) + a:2]
                            nc.tensor.matmul(fr, c1t[:, a, ks], rhs,
                                             start=(a == 0), stop=(a == 3))
                        fi = pss.tile([128, F], f32, tag="ff", name="fi")
                        for a in range(4):
                            rhs = V[:, b, 2 * t0 + a:2 * (t0 + F) + a:2]
                            nc.tensor.matmul(fi, s1t[:, a, ks], rhs,
                                             start=(a == 0), stop=(a == 3))
                        uS = wk.tile([128, F], bf16, tag="uS", name="uS")
                        nc.scalar.activation(uS, rey, AF.Copy)
                        phiS = wk.tile([128, F], bf16, tag="phiS", name="phiS")
                        nc.scalar.activation(phiS, imy, AF.Copy)
                        frS = wk.tile([128, F], bf16, tag="frS", name="frS")
                        nc.vector.tensor_copy(frS, fr)
                        fiS = wk.tile([128, F], bf16, tag="fiS", name="fiS")
                        nc.vector.tensor_copy(fiS, fi)
                        tm1 = wk.tile([128, F], bf16, tag="tm1", name="tm1")
                        tm2 = wk.tile([128, F], bf16, tag="tm2", name="tm2")
                        nc.gpsimd.tensor_mul(tm1, uS, frS)
                        nc.gpsimd.tensor_mul(tm2, phiS, fiS)
                        nc.vector.tensor_add(
                            pre[:, kc, b, 1 + t0:1 + t0 + F], tm1, tm2)
                        tm3 = wk.tile([128, F], bf16, tag="tm3", name="tm3")
                        tm4 = wk.tile([128, F], bf16, tag="tm4", name="tm4")
                        nc.vector.tensor_mul(tm3, phiS, frS)
                        nc.vector.tensor_mul(tm4, uS, fiS)
                        nc.vector.tensor_sub(
                            pim[:, kc, b, 1 + t0:1 + t0 + F], tm3, tm4)
                # OLA roll halo: col 0 <- col T (frame T-1)
                nc.vector.tensor_copy(pre[:, :, b, 0:1], pre[:, :, b, T:T + 1])
                nc.vector.tensor_copy(pim[:, :, b, 0:1], pim[:, :, b, T:T + 1])

            def out_phase(b):
                # chunk 0 last: its OLA b-part needs the halo copy, which
                # waits on the final products — all other chunks' operands
                # are ready earlier
                for off in list(range(128, T, 128)) + [0]:
                    tb = min(128, T - off)
                    po = pso.tile([128, HOP], f32, tag="po", name="po")
                    first = True
                    for kc in range(4):
                        nc.tensor.matmul(po[:tb],
                                         pre[:, kc, b, 1 + off:1 + off + tb],
                                         c2a[:, kc, :], start=first, stop=False)
                        first = False
                        nc.tensor.matmul(po[:tb],
                                         pim[:, kc, b, 1 + off:1 + off + tb],
                                         s2a[:, kc, :], start=False, stop=False)
                        nc.tensor.matmul(po[:tb],
                                         pre[:, kc, b, off:off + tb],
                                         c2b[:, kc, :], start=False, stop=False)
                        nc.tensor.matmul(po[:tb],
                                         pim[:, kc, b, off:off + tb],
                                         s2b[:, kc, :], start=False,
                                         stop=(kc == 3))
                    osb = wk.tile([128, HOP], f32, tag="osb", name="osb")
                    nc.vector.tensor_scalar(osb[:tb], po[:tb], 1.0, -1.0,
                                            ALU.min, ALU.max)
                    nc.sync.dma_start(out=OUTD[b, off:off + tb, :], in_=osb[:tb])

            def body():
                load_b(0)
                load_b(1)
                conv(0)
                spectral(0)
                conv(1)
                out_phase(0)
                spectral(1)
                out_phase(1)

            if loop_n == 1:
                body()
            else:
                with tc.For_i(0, loop_n, 1):
                    body()

    nc.compile()
    _NCS[loop_n] = nc
    return nc


def _make_in_maps(inputs):
    mats = _build_matrices()
    wts = _prep_weights(inputs)
    x = np.asarray(inputs["x"], np.float32)
    z = np.asarray(inputs["z"], np.float32).reshape(B, -1)
    xt = x.transpose(0, 2, 1)                                     # (B, 80, 800)
    xpad = np.zeros((B, D, T + 2), np.float32)
    xpad[:, :, 1:T + 1] = xt
    # xs[b, ch, r, t] = xpad[b, cin, t+k] for R = ch*120+r = k*80+cin
    xs = np.zeros((B, 2, 120, T), np.float32)
    for R in range(240):
        k, cin = R // D, R % D
        xs[:, R // 120, R % 120, :] = xpad[:, cin, k:k + T]
    zp = np.zeros((B, ZPAD), np.float32)
    zp[:, WIN // 2 - 1:WIN // 2 - 1 + T * HOP] = z
    vt = zp.reshape(B, NU, 128).transpose(0, 2, 1)                # (B, 128, NU)
    vt = np.ascontiguousarray(vt.astype(BF))
    shared = {**mats, **wts}
    in_maps = []
    for c in range(N_CORES):
        m = dict(shared)
        m["xs"] = np.ascontiguousarray(xs[BPC * c:BPC * (c + 1)])
        m["vt"] = np.ascontiguousarray(vt[BPC * c:BPC * (c + 1)])
        in_maps.append(m)
    return in_maps


def kernel(**inputs):
    nc = build_nc(loop_n=1)
    in_maps = _make_in_maps(inputs)
    res = run_bass_kernel_spmd(nc, in_maps, list(range(N_CORES)))
    out = np.concatenate([r["out"].reshape(BPC, 1, T * HOP)
                          for r in res.results], axis=0)
    return out.astype(np.float32)


# revision 19
# speedup vs baseline: 1.3630x; 1.3630x over previous
"""Trainium2 Bass kernel for nn_ConvLTVFilterGenerator (v2).

Pipeline (per batch elem, data-parallel over B across 8 cores, 2 elems/core):
  conv stack (4 conv1d k=3 layers, grouped convs as block-diag halves)
  -> cepstrum DFT (matmul vs cos/sin matrices, quef folded into W4)
  -> Z-1 ~= u + i*phi (1st-order Taylor; |u|,|phi| < 0.011 so the 2nd-order
     term is ~1e-5 relative)
  -> P = (Z-1) * conj(F) per frame, F = frame DFT via bf16 matmuls
  -> window + OLA fused into the final matmul (PSUM accumulates the t and
     t-1 halves); k=512 Nyquist bin and the identity-delta path dropped
     (together ~6e-3 relative, budget 2e-2)

Engine split per spectral iteration (kc, t0): PE 12 matmuls (4800 cyc),
Act 4 PSUM->SBUF bf16 copies, DVE 6 bf16 product ops. PE-bound by design.
"""
import sys

sys.path.insert(0, "/opt/trn_rl_repo")

import numpy as np
import ml_dtypes

import concourse.bacc as bacc
import concourse.tile as tile
from concourse import mybir
from concourse.bass_utils import run_bass_kernel_spmd

B, T, D = 16, 800, 80
HOP, WIN, FFT = 256, 512, 1024
CCH, OUT, GRP = 256, 222, 8
NB = 512                   # spectral bins kept (Nyquist dropped)
N_CORES = 8
BPC = B // N_CORES         # 2 batch elems per core
ZPAD = T * HOP + 512       # 205312 = 1604*128
NU = ZPAD // 128           # 1604
F = 400                    # frames per matmul half

f32 = mybir.dt.float32
f32r = mybir.dt.float32r
bf16 = mybir.dt.bfloat16
AF = mybir.ActivationFunctionType
ALU = mybir.AluOpType
BF = ml_dtypes.bfloat16

_MATS = None
_NCS = {}


def _build_matrices():
    """Input-independent DFT/OLA matrices, host-side fp64 -> fp32/bf16."""
    global _MATS
    if _MATS is not None:
        return _MATS
    w = 2 * np.pi / FFT
    k = np.arange(NB, dtype=np.float64)[:, None]          # (512, 1)
    c = np.arange(OUT, dtype=np.float64)[None, :]
    s_exp = np.log(10.0) / 10.0
    pad = (FFT - OUT) // 2
    CaN = np.cos(w * k * (pad + c)) * s_exp               # (512, 222)
    SaN = -np.sin(w * k * (pad + c))
    j = np.arange(WIN, dtype=np.float64)[None, :]
    C1 = np.cos(w * k * j)                                # (512, 512)
    S1 = -np.sin(w * k * j)
    n = np.arange(WIN, dtype=np.float64)
    win = 0.5 * (1.0 - np.cos(2.0 * np.pi * n / WIN))
    wk = np.full(NB, 2.0); wk[0] = 1.0
    d = (WIN - 1 - n)[None, :]
    C2 = (win[None, :] * wk[:, None] * np.cos(w * k * d)) / FFT   # (512, 512)
    S2 = (-win[None, :] * wk[:, None] * np.sin(w * k * d)) / FFT

    cat = np.zeros((128, 2, NB), np.float64)
    sat = np.zeros((128, 2, NB), np.float64)
    for ch in range(2):
        rows = min(128, OUT - 128 * ch)
        cat[:rows, ch, :] = CaN[:, 128 * ch:128 * ch + rows].T
        sat[:rows, ch, :] = SaN[:, 128 * ch:128 * ch + rows].T
    c1t = np.zeros((128, 4, NB), np.float64)
    s1t = np.zeros((128, 4, NB), np.float64)
    for a in range(4):
        c1t[:, a, :] = C1[:, 128 * a:128 * (a + 1)].T
        s1t[:, a, :] = S1[:, 128 * a:128 * (a + 1)].T
    c2a = np.zeros((128, 4, HOP), np.float64)
    c2b = np.zeros((128, 4, HOP), np.float64)
    s2a = np.zeros((128, 4, HOP), np.float64)
    s2b = np.zeros((128, 4, HOP), np.float64)
    for kc in range(4):
        c2a[:, kc, :] = C2[128 * kc:128 * (kc + 1), :HOP]
        c2b[:, kc, :] = C2[128 * kc:128 * (kc + 1), HOP:]
        s2a[:, kc, :] = S2[128 * kc:128 * (kc + 1), :HOP]
        s2b[:, kc, :] = S2[128 * kc:128 * (kc + 1), HOP:]

    def f32a(a):
        return np.ascontiguousarray(a, np.float32)

    def bfa(a):
        return np.ascontiguousarray(np.asarray(a, np.float32).astype(BF))

    _MATS = dict(
        cat=f32a(cat), sat=f32a(sat),
        c1t=bfa(c1t), s1t=bfa(s1t),
        c2a=bfa(c2a), c2b=bfa(c2b), s2a=bfa(s2a), s2b=bfa(s2b))
    return _MATS


def _prep_weights(inp):
    """Input-dependent weight rearrangements (host)."""
    idx = np.arange(1, OUT // 2 + 1, dtype=np.float64)
    quef = np.concatenate([idx[::-1], idx])
    W1 = np.asarray(inp["W1"], np.float64)
    W2 = np.asarray(inp["W2"], np.float64)
    W3 = np.asarray(inp["W3"], np.float64)
    W4 = np.asarray(inp["W4"], np.float64)
    # conv1 tap-packed: contraction row R = k*80+c (k tap, c in-channel),
    # split into two K=120 chunks
    w1s = np.zeros((120, 2, CCH), np.float64)
    for R in range(240):
        k, cin = R // D, R % D
        w1s[R % 120, R // 120, :] = W1[:, cin, k]

    def blockdiag(W):
        bd = np.zeros((128, 3, 2, 128), np.float64)
        for H in range(2):
            for o in range(128):
                g = o // 32
                for kk in range(3):
                    bd[g * 32:(g + 1) * 32, kk, H, o] = W[128 * H + o, :, kk]
        return np.ascontiguousarray(bd, np.float32)

    W4q = W4 / quef[:, None, None]
    w4t = np.zeros((128, 2, 3, OUT), np.float64)
    for cch in range(2):
        for kk in range(3):
            w4t[:, cch, kk, :] = W4q[:, 128 * cch:128 * (cch + 1), kk].T
    b4q = np.asarray(inp["b4"], np.float64) / quef

    def bias2(b):
        out = np.zeros((128, 2), np.float32)
        bb = np.asarray(b, np.float64)
        out[:, 0] = bb[:128]
        out[:len(bb) - 128, 1] = bb[128:]
        return out

    return dict(
        w1s=np.ascontiguousarray(w1s, np.float32),
        bd2=blockdiag(W2), bd3=blockdiag(W3),
        w4t=np.ascontiguousarray(w4t, np.float32),
        b1t=bias2(inp["b1"]), b2t=bias2(inp["b2"]), b3t=bias2(inp["b3"]),
        b4t=bias2(b4q))


def build_nc(loop_n=1):
    """Build + compile the per-core Bass program."""
    if loop_n in _NCS:
        return _NCS[loop_n]
    nc = bacc.Bacc("TRN2", target_bir_lowering=False, debug=False)

    def din(name, shape, dt=f32r):
        return nc.dram_tensor(name, list(shape), dt, kind="ExternalInput").ap()

    XS = din("xs", (BPC, 2, 120, T))
    VT = din("vt", (BPC, 128, NU), bf16)
    CAT = din("cat", (128, 2, NB)); SAT = din("sat", (128, 2, NB))
    C1T = din("c1t", (128, 4, NB), bf16); S1T = din("s1t", (128, 4, NB), bf16)
    C2A = din("c2a", (128, 4, HOP), bf16); C2B = din("c2b", (128, 4, HOP), bf16)
    S2A = din("s2a", (128, 4, HOP), bf16); S2B = din("s2b", (128, 4, HOP), bf16)
    W1S = din("w1s", (120, 2, CCH))
    BD2 = din("bd2", (128, 3, 2, 128)); BD3 = din("bd3", (128, 3, 2, 128))
    W4T = din("w4t", (128, 2, 3, OUT))
    B1 = nc.dram_tensor("b1t", [128, 2], f32, kind="ExternalInput").ap()
    B2 = nc.dram_tensor("b2t", [128, 2], f32, kind="ExternalInput").ap()
    B3 = nc.dram_tensor("b3t", [128, 2], f32, kind="ExternalInput").ap()
    B4 = nc.dram_tensor("b4t", [128, 2], f32, kind="ExternalInput").ap()
    OUTD = nc.dram_tensor("out", [BPC, T, HOP], f32, kind="ExternalOutput").ap()

    with tile.TileContext(nc) as tc:
        with tc.tile_pool(name="consts", bufs=1) as cst, \
             tc.tile_pool(name="work", bufs=2) as wk, \
             tc.tile_pool(name="psc", bufs=2, space="PSUM") as psc, \
             tc.tile_pool(name="pss", bufs=2, space="PSUM") as pss, \
             tc.tile_pool(name="pso", bufs=2, space="PSUM") as pso:

            def load(name, src, shape, dt=f32r):
                t = cst.tile(list(shape), dt, name=name)
                nc.sync.dma_start(out=t, in_=src)
                return t

            cat = load("catS", CAT, (128, 2, NB))
            sat = load("satS", SAT, (128, 2, NB))
            c1t = load("c1tS", C1T, (128, 4, NB), bf16)
            s1t = load("s1tS", S1T, (128, 4, NB), bf16)
            c2a = load("c2aS", C2A, (128, 4, HOP), bf16)
            c2b = load("c2bS", C2B, (128, 4, HOP), bf16)
            s2a = load("s2aS", S2A, (128, 4, HOP), bf16)
            s2b = load("s2bS", S2B, (128, 4, HOP), bf16)
            w1s = load("w1sS", W1S, (120, 2, CCH))
            bd2 = load("bd2S", BD2, (128, 3, 2, 128))
            bd3 = load("bd3S", BD3, (128, 3, 2, 128))
            w4t = load("w4tS", W4T, (128, 2, 3, OUT))
            b1t = load("b1tS", B1, (128, 2), f32)
            b2t = load("b2tS", B2, (128, 2), f32)
            b3t = load("b3tS", B3, (128, 2), f32)
            b4t = load("b4tS", B4, (128, 2), f32)

            # persistent data tiles, manually double-buffered on dim 1
            xs_sb = cst.tile([120, BPC, 2, T], f32r, name="xs_sb")
            V = cst.tile([128, BPC, NU], bf16, name="V")
            h1 = cst.tile([128, 2, T + 2], f32r, name="h1")
            h2 = cst.tile([128, 2, T + 2], f32r, name="h2")
            h3 = cst.tile([128, 2, T + 2], f32r, name="h3")
            ccep = cst.tile([128, BPC, 2, T], f32r, name="ccep")
            pre = cst.tile([128, 4, BPC, T + 1], bf16, name="pre")
            pim = cst.tile([128, 4, BPC, T + 1], bf16, name="pim")
            # zero the conv halo columns once (never rewritten)
            zb = cst.tile([128, 1], f32, name="zb")
            nc.vector.memset(zb, 0.0)
            for h in (h1, h2, h3):
                for m in range(2):
                    nc.vector.tensor_copy(h[:, m, 0:1], zb)
                    nc.vector.tensor_copy(h[:, m, T + 1:T + 2], zb)

            def load_b(b):
                for ch in range(2):
                    nc.sync.dma_start(out=xs_sb[:, b, ch, :], in_=XS[b, ch])
                nc.sync.dma_start(out=V[:, b, :], in_=VT[b])

            def relu_psum(dst, pc, bt, on_act):
                """bias-add + relu from PSUM, on Act or DVE."""
                if on_act:
                    nc.scalar.activation(dst, pc, AF.Relu, bias=bt, scale=1.0)
                else:
                    nc.vector.tensor_scalar(dst, pc, bt, 0.0, ALU.add, ALU.max)

            def conv(b):
                for t0 in (0, F):
                    for m in range(2):
                        pc = psc.tile([128, F], f32, tag="pc", name="pc1")
                        for ch in range(2):
                            nc.tensor.matmul(
                                pc, w1s[:, ch, 128 * m:128 * (m + 1)],
                                xs_sb[:, b, ch, t0:t0 + F],
                                start=(ch == 0), stop=(ch == 1))
                        relu_psum(h1[:, m, 1 + t0:1 + t0 + F], pc,
                                  b1t[:, m:m + 1], on_act=(m == 0))
                for hsrc, hdst, bdw, bt in ((h1, h2, bd2, b2t),
                                            (h2, h3, bd3, b3t)):
                    for t0 in (0, F):
                        for m in range(2):
                            pc = psc.tile([128, F], f32, tag="pc", name="pc2")
                            for kk in range(3):
                                nc.tensor.matmul(
                                    pc, bdw[:, kk, m, :],
                                    hsrc[:, m, t0 + kk:t0 + kk + F],
                                    start=(kk == 0), stop=(kk == 2))
                            relu_psum(hdst[:, m, 1 + t0:1 + t0 + F], pc,
                                      bt[:, m:m + 1], on_act=(m == 0))
                for t0 in (0, F):
                    for m in range(2):
                        sz = min(128, OUT - 128 * m)
                        pc = psc.tile([128, F], f32, tag="pc", name="pc4")
                        first = True
                        for cch in range(2):
                            for kk in range(3):
                                nc.tensor.matmul(
                                    pc[:sz], w4t[:, cch, kk, 128 * m:128 * m + sz],
                                    h3[:, cch, t0 + kk:t0 + kk + F],
                                    start=first, stop=(cch == 1 and kk == 2))
                                first = False
                        nc.vector.tensor_scalar_add(
                            ccep[:sz, b, m, t0:t0 + F], pc[:sz],
                            b4t[:sz, m:m + 1])

            def spectral(b):
                for t0 in (0, F):
                    for kc in range(4):
                        ks = slice(128 * kc, 128 * (kc + 1))
                        rey = pss.tile([128, F], f32, tag="ri", name="rey")
                        nc.tensor.matmul(rey, cat[:, 0, ks],
                                         ccep[:, b, 0, t0:t0 + F],
                                         start=True, stop=False)
                        nc.tensor.matmul(rey, cat[:94, 1, ks],
                                         ccep[:94, b, 1, t0:t0 + F],
                                         start=False, stop=True)
                        imy = pss.tile([128, F], f32, tag="ri", name="imy")
                        nc.tensor.matmul(imy, sat[:, 0, ks],
                                         ccep[:, b, 0, t0:t0 + F],
                                         start=True, stop=False)
                        nc.tensor.matmul(imy, sat[:94, 1, ks],
                                         ccep[:94, b, 1, t0:t0 + F],
                                         start=False, stop=True)
                        fr = pss.tile([128, F], f32, tag="ff", name="fr")
                        for a in range(4):
                            rhs = V[:, b, 2 * t0 + a:2 * (t0 + F) + a:2]
                            nc.tensor.matmul(fr, c1t[:, a, ks], rhs,
                                             start=(a == 0), stop=(a == 3))
                        fi = pss.tile([128, F], f32, tag="ff", name="fi")
                        for a in range(4):
                            rhs = V[:, b, 2 * t0 + a:2 * (t0 + F) + a:2]
                            nc.tensor.matmul(fi, s1t[:, a, ks], rhs,
                                             start=(a == 0), stop=(a == 3))
                        uS = wk.tile([128, F], bf16, tag="uS", name="uS")
                        nc.scalar.activation(uS, rey, AF.Copy)
                        phiS = wk.tile([128, F], bf16, tag="phiS", name="phiS")
                        nc.scalar.activation(phiS, imy, AF.Copy)
                        frS = wk.tile([128, F], bf16, tag="frS", name="frS")
                        nc.scalar.activation(frS, fr, AF.Copy)
                        fiS = wk.tile([128, F], bf16, tag="fiS", name="fiS")
                        nc.scalar.activation(fiS, fi, AF.Copy)
                        tm1 = wk.tile([128, F], bf16, tag="tm1", name="tm1")
                        tm2 = wk.tile([128, F], bf16, tag="tm2", name="tm2")
                        nc.vector.tensor_mul(tm1, uS, frS)
                        nc.vector.tensor_mul(tm2, phiS, fiS)
                        nc.vector.tensor_add(
                            pre[:, kc, b, 1 + t0:1 + t0 + F], tm1, tm2)
                        tm3 = wk.tile([128, F], bf16, tag="tm3", name="tm3")
                        tm4 = wk.tile([128, F], bf16, tag="tm4", name="tm4")
                        nc.vector.tensor_mul(tm3, phiS, frS)
                        nc.vector.tensor_mul(tm4, uS, fiS)
                        nc.vector.tensor_sub(
                            pim[:, kc, b, 1 + t0:1 + t0 + F], tm3, tm4)
                # OLA roll halo: col 0 <- col T (frame T-1)
                nc.vector.tensor_copy(pre[:, :, b, 0:1], pre[:, :, b, T:T + 1])
                nc.vector.tensor_copy(pim[:, :, b, 0:1], pim[:, :, b, T:T + 1])

            def out_phase(b):
                # chunk 0 last: its OLA b-part needs the halo copy, which
                # waits on the final products — all other chunks' operands
                # are ready earlier
                for off in list(range(128, T, 128)) + [0]:
                    tb = min(128, T - off)
                    po = pso.tile([128, HOP], f32, tag="po", name="po")
                    first = True
                    for kc in range(4):
                        nc.tensor.matmul(po[:tb],
                                         pre[:, kc, b, 1 + off:1 + off + tb],
                                         c2a[:, kc, :], start=first, stop=False)
                        first = False
                        nc.tensor.matmul(po[:tb],
                                         pim[:, kc, b, 1 + off:1 + off + tb],
                                         s2a[:, kc, :], start=False, stop=False)
                        nc.tensor.matmul(po[:tb],
                                         pre[:, kc, b, off:off + tb],
                                         c2b[:, kc, :], start=False, stop=False)
                        nc.tensor.matmul(po[:tb],
                                         pim[:, kc, b, off:off + tb],
                                         s2b[:, kc, :], start=False,
                                         stop=(kc == 3))
                    osb = wk.tile([128, HOP], f32, tag="osb", name="osb")
                    nc.vector.tensor_scalar(osb[:tb], po[:tb], 1.0, -1.0,
                                            ALU.min, ALU.max)
                    nc.sync.dma_start(out=OUTD[b, off:off + tb, :], in_=osb[:tb])

            def body():
                load_b(0)
                load_b(1)
                conv(0)
                spectral(0)
                conv(1)
                out_phase(0)
                spectral(1)
                out_phase(1)

            if loop_n == 1:
                body()
            else:
                with tc.For_i(0, loop_n, 1):
                    body()

    nc.compile()
    _NCS[loop_n] = nc
    return nc


def _make_in_maps(inputs):
    mats = _build_matrices()
    wts = _prep_weights(inputs)
    x = np.asarray(inputs["x"], np.float32)
    z = np.asarray(inputs["z"], np.float32).reshape(B, -1)
    xt = x.transpose(0, 2, 1)                                     # (B, 80, 800)
    xpad = np.zeros((B, D, T + 2), np.float32)
    xpad[:, :, 1:T + 1] = xt
    # xs[b, ch, r, t] = xpad[b, cin, t+k] for R = ch*120+r = k*80+cin
    xs = np.zeros((B, 2, 120, T), np.float32)
    for R in range(240):
        k, cin = R // D, R % D
        xs[:, R // 120, R % 120, :] = xpad[:, cin, k:k + T]
    zp = np.zeros((B, ZPAD), np.float32)
    zp[:, WIN // 2 - 1:WIN // 2 - 1 + T * HOP] = z
    vt = zp.reshape(B, NU, 128).transpose(0, 2, 1)                # (B, 128, NU)
    vt = np.ascontiguousarray(vt.astype(BF))
    shared = {**mats, **wts}
    in_maps = []
    for c in range(N_CORES):
        m = dict(shared)
        m["xs"] = np.ascontiguousarray(xs[BPC * c:BPC * (c + 1)])
        m["vt"] = np.ascontiguousarray(vt[BPC * c:BPC * (c + 1)])
        in_maps.append(m)
    return in_maps


def kernel(**inputs):
    nc = build_nc(loop_n=1)
    in_maps = _make_in_maps(inputs)
    res = run_bass_kernel_spmd(nc, in_maps, list(range(N_CORES)))
    out = np.concatenate([r["out"].reshape(BPC, 1, T * HOP)
                          for r in res.results], axis=0)
    return out.astype(np.float32)
